# revision 21
# baseline (speedup 1.0000x reference)
"""Trainium2 Bass kernel: batched single-head self-attention.

Reference computation (per (b, l) pair, 20 independent blocks):
    X = x[b, l] viewed as [N=1024, D=256] (xf layout)
    out[b, l] = softmax(beta * X @ X.T, axis=-1) @ X

Device algorithm (per block):
  * Scores: S[m, n] = sum_d X^T[d, m] X^T[d, n] on the TensorEngine with
    D on partitions -- the natural HBM layout of x[b, l] is already X^T.
    S is symmetric, so the PSUM tile doubles as the [keys, queries]
    orientation the second matmul wants: no transpose of the score
    matrix, ever.
  * Softmax shift: W[m, n] = exp(beta * (S[m, n] - c_n)) with
    c_n = ||x_n||^2 (the score diagonal -- a valid shift here since the
    attention is diagonal-dominant by ~100 nats). The per-QUERY shift is
    PSUM-PRELOADED: a per-slab broadcast tile bc[p, n] = -c_n (built once
    by a tiny ones x negc matmul, evacuated to SBUF) is copied by the
    DVE into each score PSUM tile before the score matmuls accumulate
    onto it (start=False). This moves the shift entirely off the
    TensorEngine stream (the old K=1 ride cost a full extra pass,
    8192 cycles/block).
  * Second matmul: computed as O^T[d, n] = sum_m xfo[m, d] W[m, n] with
    the value operand xfo = [X | 1 | 0] STATIONARY. The [1|0] chunk
    makes the softmax denominator Z_n fall out as an extra 2-row matmul
    riding the same PSUM accumulation loop (interleaved per key tile so
    there is no post-loop PE tail). Normalization (divide by Z) and the
    final [d, n] -> [n, d] layout flip happen on the host, where they
    are free.
  * Everything runs in fp32r (relaxed fp32: ~13-bit effective mantissa,
    full-rate 1 col/cycle PE streaming vs 4 cyc/col for exact fp32).
    bf16 scores are NOT enough (near-duplicate key pairs), fp32r is.

Pipelining / engine budget (per core, ~2.5 blocks):
  * PE: 2 passes scores + 2 passes O^T + 1 pass Z = 5 x 8192 cycles per
    block ~= 43 us at 2.4 GHz. Everything else hides under it.
  * ACT: exp (40 tiles x ~0.7us) + half the O^T evacuation.
  * DVE: bias preloads (40 x ~0.7us) + bc builds + Z/O^T evacuation.
  * Input DMA: slab 0's score operand is split across BOTH hw queues so
    the first matmul can start ~3 us earlier; remaining slabs stream
    under compute. Outputs go out on the gpsimd queue (s<2) and split
    across the sync+scalar queues for the final half slab (short tail).
  * PE p-state ramp: a few throwaway full-width fp32r matmuls run
    during the input-DMA window so the clock ramp (~3 us) mostly
    completes before real data arrives.

Sharding: 20 blocks over 8 cores as 2 full blocks + 1 half block (512
queries) per core -- exact, no padded compute. The half blocks use a
host-side rotation of the key axis so every core runs the identical
program (softmax is invariant to key permutation when values are
permuted identically).
"""

import numpy as np
import ml_dtypes

import concourse.tile as tile
from concourse import bacc, mybir
from concourse.bass_utils import run_bass_kernel_spmd

F32 = mybir.dt.float32
F32R = mybir.dt.float32r
BF16 = mybir.dt.bfloat16

B, L, D, H, W = 4, 5, 256, 32, 32
N = H * W            # 1024 keys per block
NBLK = B * L         # 20
NCORES = 8
NFULL = 2            # full blocks per core
NSLAB = 3            # 2 full + 1 half
DF = D + 8           # value operand row: [x | 1 | 0 | pad...] -- padded to
                     # 264 floats = 1056 B so SBUF rows stay 32B-aligned
                     # (unaligned weight rows double LDWEIGHTS time)
NWARM = 1            # PE clock-ramp warmup matmuls (the bias
                     # builds continue the ramp with real work)

EXP = mybir.ActivationFunctionType.Exp


def build_program(beta: float, fast: bool = True):
    mdt = F32R if fast else F32   # all matmul operands
    nc = bacc.Bacc("TRN2", target_bir_lowering=False, debug=False,
                   num_devices=NCORES)
    # Inputs are host-packed in device layout so every DMA is a plain
    # contiguous [128, *] transfer with large descriptors.
    xb_in = nc.dram_tensor("xb_in", [NSLAB, 128, 2, N], mdt,
                           kind="ExternalInput")
    xf_in = nc.dram_tensor("xf_in", [NSLAB, 128, 8, DF], mdt,
                           kind="ExternalInput")
    nc_in = nc.dram_tensor("nc_in", [1, NSLAB * N], mdt, kind="ExternalInput")
    yt_out = nc.dram_tensor("yt_out", [NSLAB, 128, 2, N], F32,
                            kind="ExternalOutput")
    z_out = nc.dram_tensor("z_out", [NSLAB, N], F32, kind="ExternalOutput")

    with tile.TileContext(nc) as tc:
        _build(tc, nc, xb_in.ap(), xf_in.ap(), nc_in.ap(), yt_out.ap(),
               z_out.ap(), beta, mdt)
    nc.finalize()
    return nc


def _build(tc, nc, xb_in, xf_in, nc_in, yt_out, z_out, beta, mdt):
    import contextlib
    ctx = contextlib.ExitStack()
    with ctx:
        const = ctx.enter_context(tc.tile_pool(name="const", bufs=1))
        xb_pool = ctx.enter_context(tc.tile_pool(name="xb", bufs=NSLAB))
        xfo_pool = ctx.enter_context(tc.tile_pool(name="xfo", bufs=NSLAB))
        negc_pool = ctx.enter_context(tc.tile_pool(name="negc", bufs=1))
        bc_pool = ctx.enter_context(tc.tile_pool(name="bc", bufs=NSLAB))
        # W tiles stay live until the post-loop Z pass of their slab.
        w_pool = ctx.enter_context(tc.tile_pool(name="w", bufs=9))
        ot_sb_pool = ctx.enter_context(tc.tile_pool(name="ot_sb", bufs=2))
        z_sb_pool = ctx.enter_context(tc.tile_pool(name="z_sb", bufs=2))
        # PSUM budget (8 banks of [128, 512] f32):
        #   4 score tiles + 4 O^T accumulators. The bias preloads get a
        #   full key-tile of slack from the 4-deep score rotation; the Z
        #   row-sum pass runs post-loop in freed score banks (full
        #   slabs) or interleaved in a spare O^T slot (final half slab).
        ps_s = ctx.enter_context(tc.tile_pool(name="ps_s", bufs=4, space="PSUM"))
        ps_od = ctx.enter_context(tc.tile_pool(name="ps_od", bufs=4, space="PSUM"))


        # Warm the PE clock (HAM) with throwaway full-array matmuls that
        # run during the input-DMA window -- otherwise the first ~3us of
        # real matmuls run at half clock.
        warm_src = const.tile([128, 512], F32)
        nc.gpsimd.memset(warm_src[:], 0.0)
        warm_ps = ps_od.tile([128, 512], F32, tag="od", name="warm_ps")
        for wi in range(NWARM):
            nc.tensor.matmul(warm_ps[:], warm_src[:, 0:128], warm_src[:],
                             start=True, stop=True)

        # Input DMAs: slab 0's score operand is split across both hw
        # queues (earliest possible first matmul); the rest interleave so
        # each slab's operands land well before its compute window. The
        # bias broadcast tiles bc[p, n] = -c_n are built by partition-
        # broadcast DMAs (stride-0 source) on the otherwise-idle gpsimd
        # queue -- no compute engine touches them.
        xbs = [xb_pool.tile([128, 2, N], mdt, tag="xb", name=f"xb_{s}")
               for s in range(NSLAB)]
        xfos = [xfo_pool.tile([128, 8, DF], mdt, tag="xfo", name=f"xfo_{s}")
                for s in range(NSLAB)]
        bcs = [bc_pool.tile([128, N], F32, tag="bc", name=f"bc_{s}")
               for s in range(NSLAB)]
        negc_all = negc_pool.tile([1, NSLAB * N], mdt, tag="negc")
        # sync queue (starts flowing ~2us earlier than scalar; carries the
        # bias row -- which gates the first preload -- then chunk 0 and
        # the high-column half of chunk 1)
        nc.sync.dma_start(out=negc_all[:], in_=nc_in[:])
        nc.sync.dma_start(out=xbs[0][:, 0:1, :], in_=xb_in[0][:, 0:1, :])
        nc.sync.dma_start(out=xbs[0][:, 1:2, 512:N],
                          in_=xb_in[0][:, 1:2, 512:N])
        nc.sync.dma_start(out=xfos[0][:, 0:4, :], in_=xf_in[0][:, 0:4, :])
        nc.sync.dma_start(out=xbs[1][:], in_=xb_in[1])
        nc.sync.dma_start(out=xbs[2][:], in_=xb_in[2])
        # scalar queue (low half of chunk 1 first: it feeds the h0 scores
        # and the early key-tile weights)
        nc.scalar.dma_start(out=xbs[0][:, 1:2, 0:512],
                            in_=xb_in[0][:, 1:2, 0:512])
        nc.scalar.dma_start(out=xfos[0][:, 4:8, :], in_=xf_in[0][:, 4:8, :])
        nc.scalar.dma_start(out=xfos[1][:], in_=xf_in[1])
        nc.scalar.dma_start(out=xfos[2][:], in_=xf_in[2])

        ones_col_f32 = const.tile([1, 128], F32)
        nc.gpsimd.memset(ones_col_f32[:], 1.0)
        if mdt is F32:
            ones_col = ones_col_f32
        else:
            ones_col = const.tile([1, 128], mdt)
            nc.vector.tensor_copy(ones_col[:], ones_col_f32[:])

        # Bias broadcast tiles bc[p, n] = -c_n for all slabs, built
        # upfront during the DMA window: K=1 ones x negc matmuls into
        # transient score-pool banks, evacuated to SBUF by the DVE (the
        # first preload chains directly off the slab-0 h0 copy).
        for s in range(NSLAB):
            for h in range(2 if s < NFULL else 1):
                hs = slice(h * 512, (h + 1) * 512)
                bc_ps = ps_s.tile([128, 512], F32, tag="sps",
                                  name=f"bcps_{s}_{h}")
                nc.tensor.matmul(bc_ps[:], ones_col[:],
                                 negc_all[:, s * N + h * 512:
                                          s * N + (h + 1) * 512],
                                 start=True, stop=True)
                nc.vector.tensor_copy(bcs[s][:, hs], bc_ps[:])
        nc.tensor.matmul(warm_ps[:], warm_src[:, 0:128], warm_src[:],
                         start=True, stop=True)

        pending_z = []
        for s in range(NSLAB):
            n_q = N if s < NFULL else N // 2
            n_h = n_q // 512    # PSUM bank halves (queries)
            xb, xfo, bc = xbs[s], xfos[s], bcs[s]

            # O^T accumulators, live across the whole key loop; the
            # final half slab also carves a Z accumulator out of the
            # spare O^T slots.
            od = [[ps_od.tile([128, 512], F32, tag="od",
                              name=f"od_{s}_{ci}_{h}")
                   for h in range(n_h)] for ci in range(2)]
            oz_il = (None if s < NFULL else
                     ps_od.tile([128, 512], F32, tag="od", name=f"oz_{s}"))

            w_tiles = []
            for a in range(8):      # key tile (partitions of S' and W)
                if a == 1 and pending_z:
                    # previous slab's Z pass, deferred past this slab's
                    # first key tile: its scores+exp fill the PE->ACT
                    # pipeline while the Z matmuls run, so the slab
                    # boundary has no drain bubble
                    for fn in pending_z:
                        fn()
                    pending_z = []
                asl = slice(a * 128, (a + 1) * 128)
                wt = w_pool.tile([128, N], mdt, tag="w", name=f"w_{s}_{a}")
                sps = []
                for h in range(n_h):
                    t = ps_s.tile([128, 512], F32, tag="sps",
                                  name=f"sps_{s}_{a}_{h}")
                    # bias preload: PSUM starts at -c_n, scores accumulate
                    nc.vector.tensor_copy(t[:], bc[:, h * 512:(h + 1) * 512])
                    sps.append(t)
                # S' = -c_n + S: chunk 0 streams both query halves per
                # weight load, then chunk 1 with exp chasing each half.
                for h in range(n_h):
                    nc.tensor.matmul(sps[h][:], xb[:, 0, asl],
                                     xb[:, 0, h * 512:(h + 1) * 512],
                                     start=False, stop=False,
                                     skip_group_check=True)
                for h in range(n_h):
                    hs = slice(h * 512, (h + 1) * 512)
                    nc.tensor.matmul(sps[h][:], xb[:, 1, asl],
                                     xb[:, 1, hs],
                                     start=False, stop=True,
                                     skip_group_check=True)
                    # W = exp(beta * S'), one ACT pass per query half
                    nc.scalar.activation(wt[:, hs], sps[h][:], EXP,
                                         scale=float(beta))
                # final half slab: Z rides the key loop (no PE tail);
                # it goes first so the z evacuation chain starts earliest
                if s >= NFULL:
                    nc.tensor.matmul(oz_il[0:2, 0:512],
                                     xfo[:, a, 256:258], wt[:, 0:512],
                                     start=(a == 0), stop=(a == 7))
                # O^T += xfo[a].T @ W[a]  (value operand stationary)
                for ci, csl in ((0, slice(0, 128)), (1, slice(128, 256))):
                    for h in range(n_h):
                        hs = slice(h * 512, (h + 1) * 512)
                        nc.tensor.matmul(od[ci][h][:], xfo[:, a, csl],
                                         wt[:, hs],
                                         start=(a == 0), stop=(a == 7))
                w_tiles.append(wt)

            if s < NFULL:
                # O^T evacuation (DVE + ACT in parallel) and shipping
                # happen now; the Z pass (row sums of W in freed score
                # banks) is deferred into the next slab's stream.
                ot_sb = ot_sb_pool.tile([128, 2, N], F32, tag="ot_sb")
                for h in range(n_h):
                    hs = slice(h * 512, (h + 1) * 512)
                    nc.vector.tensor_copy(ot_sb[:, 0, hs], od[0][h][:])
                    nc.scalar.copy(ot_sb[:, 1, hs], od[1][h][:])
                nc.gpsimd.dma_start(out=yt_out[s], in_=ot_sb[:])

                def _flush_z(s=s, xfo=xfo, w_tiles=w_tiles, n_q=n_q,
                             n_h=n_h):
                    z_sb = z_sb_pool.tile([1, N], F32, tag="z_sb")
                    oz = [ps_s.tile([128, 512], F32, tag="sps",
                                    name=f"oz_{s}_{h}")
                          for h in range(n_h)]
                    for a in range(8):
                        for h in range(n_h):
                            hs = slice(h * 512, (h + 1) * 512)
                            nc.tensor.matmul(oz[h][0:2, 0:512],
                                             xfo[:, a, 256:258],
                                             w_tiles[a][:, hs],
                                             start=(a == 0), stop=(a == 7))
                    for h in range(n_h):
                        hs = slice(h * 512, (h + 1) * 512)
                        nc.vector.tensor_copy(z_sb[:, hs],
                                              oz[h][0:1, 0:512])
                    nc.gpsimd.dma_start(out=z_out[s][:n_q].unsqueeze(0),
                                        in_=z_sb[:, :n_q])
                pending_z.append(_flush_z)
            else:
                # final half slab: parallel evacuation (DVE + ACT), then
                # split the tail DMA across both hw queues.
                ot_sb = ot_sb_pool.tile([128, 2, N], F32, tag="ot_sb")
                z_sb = z_sb_pool.tile([1, N], F32, tag="z_sb")
                # z chain first on ACT (it has the longest DMA latency),
                # O^T halves split across DVE/ACT and both hw queues
                nc.scalar.copy(z_sb[0:1, 0:512], oz_il[0:1, 0:512])
                nc.vector.tensor_copy(ot_sb[:, 0, 0:512], od[0][0][:])
                nc.scalar.copy(ot_sb[:, 1, 0:512], od[1][0][:])
                nc.sync.dma_start(out=z_out[s][:n_q].unsqueeze(0),
                                  in_=z_sb[:, :n_q])
                nc.sync.dma_start(out=yt_out[s][:, 0, 0:512],
                                  in_=ot_sb[:, 0, 0:512])
                nc.scalar.dma_start(out=yt_out[s][:, 1, 0:512],
                                    in_=ot_sb[:, 1, 0:512])


_PROG_CACHE = {}


def _get_program(beta: float, fast: bool = True):
    key = (beta, fast)
    if key not in _PROG_CACHE:
        _PROG_CACHE[key] = build_program(beta, fast)
    return _PROG_CACHE[key]


def make_in_maps(x: np.ndarray, fast: bool = True):
    """Shard the full input [B, L, D, H, W] into 8 per-core input maps."""
    xt_all = np.ascontiguousarray(x.reshape(NBLK, D, N))
    in_maps = []
    for c in range(NCORES):
        half_blk = NFULL * NCORES + c // 2
        half = xt_all[half_blk]
        if c % 2 == 1:
            # rotate keys so this core's queries are columns 0..511
            half = np.concatenate([half[:, N // 2:], half[:, :N // 2]], axis=1)
        slabs = np.stack([xt_all[NFULL * c], xt_all[NFULL * c + 1], half])
        xf = np.zeros((NSLAB, N, DF), np.float32)
        xf[:, :, :D] = slabs.transpose(0, 2, 1)
        xf[:, :, D] = 1.0
        negc = -np.einsum('sdn,sdn->sn', slabs, slabs)
        # pack into device layout: xb [128, 2, N], xf [128, 8, DF]
        xb_p = slabs.reshape(NSLAB, 2, 128, N).transpose(0, 2, 1, 3)
        xf_p = xf.reshape(NSLAB, 8, 128, DF).transpose(0, 2, 1, 3)
        in_maps.append({"xb_in": np.ascontiguousarray(xb_p),
                        "xf_in": np.ascontiguousarray(xf_p),
                        "nc_in": np.ascontiguousarray(
                            negc.reshape(1, NSLAB * N))})
    return in_maps


def assemble_output(results):
    """Normalize, transpose and gather per-core outputs into [B, L, N, D]."""
    out = np.empty((NBLK, N, D), np.float32)
    for c in range(NCORES):
        yt = results[c]["yt_out"]          # [NSLAB, 128, 2, N]
        z = results[c]["z_out"]
        for s, blk, lo, n_q in ((0, NFULL * c, 0, N),
                                (1, NFULL * c + 1, 0, N),
                                (2, NFULL * NCORES + c // 2,
                                 (c % 2) * (N // 2), N // 2)):
            # O^T rows are ci*128 + p
            ot = yt[s].transpose(1, 0, 2).reshape(D, N)[:, :n_q]
            out[blk, lo:lo + n_q] = (ot / z[s, :n_q]).T
    return out.reshape(B, L, N, D)


def kernel(x, beta, _trace=False, _fast=True):
    x = np.asarray(x, dtype=np.float32)
    assert x.shape == (B, L, D, H, W), x.shape
    beta_f = float(np.asarray(beta))
    prog = _get_program(beta_f, _fast)
    in_maps = make_in_maps(x, _fast)
    res = run_bass_kernel_spmd(prog, in_maps, core_ids=list(range(NCORES)),
                               trace=_trace)
    out = assemble_output(res.results)
    if _trace:
        return out, res
    return out


# revision 22
# speedup vs baseline: 1.2012x; 1.2012x over previous
"""Trainium2 Bass kernel: batched single-head self-attention.

Reference computation (per (b, l) pair, 20 independent blocks):
    X = x[b, l] viewed as [N=1024, D=256] (xf layout)
    out[b, l] = softmax(beta * X @ X.T, axis=-1) @ X

Device algorithm (per block):
  * Scores: S[m, n] = sum_d X^T[d, m] X^T[d, n] on the TensorEngine with
    D on partitions -- the natural HBM layout of x[b, l] is already X^T.
    S is symmetric, so the PSUM tile doubles as the [keys, queries]
    orientation the second matmul wants: no transpose of the score
    matrix, ever.
  * Softmax shift: W[m, n] = exp(beta * (S[m, n] - c_n)) with
    c_n = ||x_n||^2 (the score diagonal -- a valid shift here since the
    attention is diagonal-dominant by ~100 nats). The per-QUERY shift is
    PSUM-PRELOADED: a per-slab broadcast tile bc[p, n] = -c_n (built once
    by a tiny ones x negc matmul, evacuated to SBUF) is copied by the
    DVE into each score PSUM tile before the score matmuls accumulate
    onto it (start=False). This moves the shift entirely off the
    TensorEngine stream (the old K=1 ride cost a full extra pass,
    8192 cycles/block).
  * Second matmul: computed as O^T[d, n] = sum_m xfo[m, d] W[m, n] with
    the value operand xfo = [X | 1 | 0] STATIONARY. The [1|0] chunk
    makes the softmax denominator Z_n fall out as an extra 2-row matmul
    riding the same PSUM accumulation loop (interleaved per key tile so
    there is no post-loop PE tail). Normalization (divide by Z) and the
    final [d, n] -> [n, d] layout flip happen on the host, where they
    are free.
  * Everything runs in fp32r (relaxed fp32: ~13-bit effective mantissa,
    full-rate 1 col/cycle PE streaming vs 4 cyc/col for exact fp32).
    bf16 scores are NOT enough (near-duplicate key pairs), fp32r is.

Pipelining / engine budget (per core, ~2.5 blocks):
  * PE: 2 passes scores + 2 passes O^T + 1 pass Z = 5 x 8192 cycles per
    block ~= 43 us at 2.4 GHz. Everything else hides under it.
  * ACT: exp (40 tiles x ~0.7us) + half the O^T evacuation.
  * DVE: bias preloads (40 x ~0.7us) + bc builds + Z/O^T evacuation.
  * Input DMA: slab 0's score operand is split across BOTH hw queues so
    the first matmul can start ~3 us earlier; remaining slabs stream
    under compute. Outputs go out on the gpsimd queue (s<2) and split
    across the sync+scalar queues for the final half slab (short tail).
  * PE p-state ramp: a few throwaway full-width fp32r matmuls run
    during the input-DMA window so the clock ramp (~3 us) mostly
    completes before real data arrives.

Sharding: 20 blocks over 8 cores as 2 full blocks + 1 half block (512
queries) per core -- exact, no padded compute. The half blocks use a
host-side rotation of the key axis so every core runs the identical
program (softmax is invariant to key permutation when values are
permuted identically).
"""

import numpy as np
import ml_dtypes

import concourse.tile as tile
from concourse import bacc, mybir
from concourse.bass_utils import run_bass_kernel_spmd

F32 = mybir.dt.float32
F32R = mybir.dt.float32r
BF16 = mybir.dt.bfloat16

B, L, D, H, W = 4, 5, 256, 32, 32
N = H * W            # 1024 keys per block
NBLK = B * L         # 20
NCORES = 8
NFULL = 2            # full blocks per core
NSLAB = 3            # 2 full + 1 half
DF = D + 8           # value operand row: [x | 1 | 0 | pad...] -- padded to
                     # 264 floats = 1056 B so SBUF rows stay 32B-aligned
                     # (unaligned weight rows double LDWEIGHTS time)
NWARM = 1            # PE clock-ramp warmup matmuls (the bias
                     # builds continue the ramp with real work)

EXP = mybir.ActivationFunctionType.Exp


def build_program(beta: float, fast: bool = True):
    mdt = F32R if fast else F32   # all matmul operands
    nc = bacc.Bacc("TRN2", target_bir_lowering=False, debug=False,
                   num_devices=NCORES)
    # Inputs are host-packed in device layout so every DMA is a plain
    # contiguous [128, *] transfer with large descriptors.
    xb_in = nc.dram_tensor("xb_in", [NSLAB, 128, 2, N], mdt,
                           kind="ExternalInput")
    xf_in = nc.dram_tensor("xf_in", [NSLAB, 128, 8, DF], mdt,
                           kind="ExternalInput")
    nc_in = nc.dram_tensor("nc_in", [1, NSLAB * N], mdt, kind="ExternalInput")
    yt_out = nc.dram_tensor("yt_out", [NSLAB, 128, 2, N], F32,
                            kind="ExternalOutput")
    z_out = nc.dram_tensor("z_out", [NSLAB, N], F32, kind="ExternalOutput")

    with tile.TileContext(nc) as tc:
        _build(tc, nc, xb_in.ap(), xf_in.ap(), nc_in.ap(), yt_out.ap(),
               z_out.ap(), beta, mdt)
    nc.finalize()
    return nc


def _build(tc, nc, xb_in, xf_in, nc_in, yt_out, z_out, beta, mdt):
    import contextlib
    ctx = contextlib.ExitStack()
    with ctx:
        const = ctx.enter_context(tc.tile_pool(name="const", bufs=1))
        xb_pool = ctx.enter_context(tc.tile_pool(name="xb", bufs=NSLAB))
        xfo_pool = ctx.enter_context(tc.tile_pool(name="xfo", bufs=NSLAB))
        negc_pool = ctx.enter_context(tc.tile_pool(name="negc", bufs=1))
        bc_pool = ctx.enter_context(tc.tile_pool(name="bc", bufs=NSLAB))
        # W tiles stay live until the post-loop Z pass of their slab.
        w_pool = ctx.enter_context(tc.tile_pool(name="w", bufs=9))
        ot_sb_pool = ctx.enter_context(tc.tile_pool(name="ot_sb", bufs=2))
        z_sb_pool = ctx.enter_context(tc.tile_pool(name="z_sb", bufs=2))
        # PSUM budget (8 banks of [128, 512] f32):
        #   4 score tiles + 4 O^T accumulators. The bias preloads get a
        #   full key-tile of slack from the 4-deep score rotation; the Z
        #   row-sum pass runs post-loop in freed score banks (full
        #   slabs) or interleaved in a spare O^T slot (final half slab).
        ps_s = ctx.enter_context(tc.tile_pool(name="ps_s", bufs=4, space="PSUM"))
        ps_od = ctx.enter_context(tc.tile_pool(name="ps_od", bufs=4, space="PSUM"))


        # Warm the PE clock (HAM) with throwaway full-array matmuls that
        # run during the input-DMA window -- otherwise the first ~3us of
        # real matmuls run at half clock.
        warm_src = const.tile([128, 512], F32)
        nc.gpsimd.memset(warm_src[:], 0.0)
        warm_ps = ps_od.tile([128, 512], F32, tag="od", name="warm_ps")
        for wi in range(NWARM):
            nc.tensor.matmul(warm_ps[:], warm_src[:, 0:128], warm_src[:],
                             start=True, stop=True)

        # Input DMAs: slab 0's score operand is split across both hw
        # queues (earliest possible first matmul); the rest interleave so
        # each slab's operands land well before its compute window. The
        # bias broadcast tiles bc[p, n] = -c_n are built by partition-
        # broadcast DMAs (stride-0 source) on the otherwise-idle gpsimd
        # queue -- no compute engine touches them.
        xbs = [xb_pool.tile([128, 2, N], mdt, tag="xb", name=f"xb_{s}")
               for s in range(NSLAB)]
        xfos = [xfo_pool.tile([128, 8, DF], mdt, tag="xfo", name=f"xfo_{s}")
                for s in range(NSLAB)]
        bcs = [bc_pool.tile([128, N], F32, tag="bc", name=f"bc_{s}")
               for s in range(NSLAB)]
        negc_all = negc_pool.tile([1, NSLAB * N], mdt, tag="negc")
        # sync queue (starts flowing ~2us earlier than scalar; carries the
        # bias row -- which gates the first preload -- then chunk 0 and
        # the high-column half of chunk 1)
        nc.sync.dma_start(out=negc_all[:], in_=nc_in[:])
        nc.sync.dma_start(out=xbs[0][:, 0:1, :], in_=xb_in[0][:, 0:1, :])
        nc.sync.dma_start(out=xfos[0][:, 0:4, :], in_=xf_in[0][:, 0:4, :])
        nc.sync.dma_start(out=xbs[1][:], in_=xb_in[1])
        nc.sync.dma_start(out=xbs[2][:], in_=xb_in[2])
        # scalar queue
        nc.scalar.dma_start(out=xbs[0][:, 1:2, :], in_=xb_in[0][:, 1:2, :])
        nc.scalar.dma_start(out=xfos[0][:, 4:8, :], in_=xf_in[0][:, 4:8, :])
        nc.scalar.dma_start(out=xfos[1][:], in_=xf_in[1])
        nc.scalar.dma_start(out=xfos[2][:], in_=xf_in[2])

        ones_col_f32 = const.tile([1, 128], F32)
        nc.gpsimd.memset(ones_col_f32[:], 1.0)
        if mdt is F32:
            ones_col = ones_col_f32
        else:
            ones_col = const.tile([1, 128], mdt)
            nc.vector.tensor_copy(ones_col[:], ones_col_f32[:])

        # Bias broadcast tiles bc[p, n] = -c_n for all slabs, built
        # upfront during the DMA window: K=1 ones x negc matmuls into
        # transient score-pool banks, evacuated to SBUF by the DVE (the
        # first preload chains directly off the slab-0 h0 copy).
        for s in range(NSLAB):
            for h in range(2 if s < NFULL else 1):
                hs = slice(h * 512, (h + 1) * 512)
                bc_ps = ps_s.tile([128, 512], F32, tag="sps",
                                  name=f"bcps_{s}_{h}")
                nc.tensor.matmul(bc_ps[:], ones_col[:],
                                 negc_all[:, s * N + h * 512:
                                          s * N + (h + 1) * 512],
                                 start=True, stop=True)
                nc.vector.tensor_copy(bcs[s][:, hs], bc_ps[:])
        nc.tensor.matmul(warm_ps[:], warm_src[:, 0:128], warm_src[:],
                         start=True, stop=True)

        pending_z = []
        for s in range(NSLAB):
            n_q = N if s < NFULL else N // 2
            n_h = n_q // 512    # PSUM bank halves (queries)
            xb, xfo, bc = xbs[s], xfos[s], bcs[s]

            # O^T accumulators, live across the whole key loop; the
            # final half slab also carves a Z accumulator out of the
            # spare O^T slots.
            od = [[ps_od.tile([128, 512], F32, tag="od",
                              name=f"od_{s}_{ci}_{h}")
                   for h in range(n_h)] for ci in range(2)]
            oz_il = (None if s < NFULL else
                     ps_od.tile([128, 512], F32, tag="od", name=f"oz_{s}"))

            w_tiles = []
            for a in range(8):      # key tile (partitions of S' and W)
                if a == 1 and pending_z:
                    # previous slab's Z pass, deferred past this slab's
                    # first key tile: its scores+exp fill the PE->ACT
                    # pipeline while the Z matmuls run, so the slab
                    # boundary has no drain bubble
                    for fn in pending_z:
                        fn()
                    pending_z = []
                asl = slice(a * 128, (a + 1) * 128)
                wt = w_pool.tile([128, N], mdt, tag="w", name=f"w_{s}_{a}")
                sps = []
                for h in range(n_h):
                    t = ps_s.tile([128, 512], F32, tag="sps",
                                  name=f"sps_{s}_{a}_{h}")
                    # bias preload: PSUM starts at -c_n, scores accumulate
                    nc.vector.tensor_copy(t[:], bc[:, h * 512:(h + 1) * 512])
                    sps.append(t)
                # S' = -c_n + S, per query half; order c0,c1 within a half
                # so exp(h) can issue before the other half's matmuls.
                for h in range(n_h):
                    hs = slice(h * 512, (h + 1) * 512)
                    for c in range(2):
                        nc.tensor.matmul(sps[h][:], xb[:, c, asl],
                                         xb[:, c, hs],
                                         start=False, stop=(c == 1),
                                         skip_group_check=True)
                    # W = exp(beta * S'), one ACT pass per query half
                    nc.scalar.activation(wt[:, hs], sps[h][:], EXP,
                                         scale=float(beta))
                # final half slab: Z rides the key loop (no PE tail);
                # it goes first so the z evacuation chain starts earliest
                if s >= NFULL:
                    nc.tensor.matmul(oz_il[0:2, 0:512],
                                     xfo[:, a, 256:258], wt[:, 0:512],
                                     start=(a == 0), stop=(a == 7))
                # O^T += xfo[a].T @ W[a]  (value operand stationary)
                for ci, csl in ((0, slice(0, 128)), (1, slice(128, 256))):
                    for h in range(n_h):
                        hs = slice(h * 512, (h + 1) * 512)
                        nc.tensor.matmul(od[ci][h][:], xfo[:, a, csl],
                                         wt[:, hs],
                                         start=(a == 0), stop=(a == 7))
                w_tiles.append(wt)

            if s < NFULL:
                # O^T evacuation (DVE + ACT in parallel) and shipping
                # happen now; the Z pass (row sums of W in freed score
                # banks) is deferred into the next slab's stream.
                ot_sb = ot_sb_pool.tile([128, 2, N], F32, tag="ot_sb")
                for h in range(n_h):
                    hs = slice(h * 512, (h + 1) * 512)
                    nc.vector.tensor_copy(ot_sb[:, 0, hs], od[0][h][:])
                    nc.scalar.copy(ot_sb[:, 1, hs], od[1][h][:])
                nc.gpsimd.dma_start(out=yt_out[s], in_=ot_sb[:])

                def _flush_z(s=s, xfo=xfo, w_tiles=w_tiles, n_q=n_q,
                             n_h=n_h):
                    z_sb = z_sb_pool.tile([1, N], F32, tag="z_sb")
                    oz = [ps_s.tile([128, 512], F32, tag="sps",
                                    name=f"oz_{s}_{h}")
                          for h in range(n_h)]
                    for a in range(8):
                        for h in range(n_h):
                            hs = slice(h * 512, (h + 1) * 512)
                            nc.tensor.matmul(oz[h][0:2, 0:512],
                                             xfo[:, a, 256:258],
                                             w_tiles[a][:, hs],
                                             start=(a == 0), stop=(a == 7))
                    for h in range(n_h):
                        hs = slice(h * 512, (h + 1) * 512)
                        nc.vector.tensor_copy(z_sb[:, hs],
                                              oz[h][0:1, 0:512])
                    nc.gpsimd.dma_start(out=z_out[s][:n_q].unsqueeze(0),
                                        in_=z_sb[:, :n_q])
                pending_z.append(_flush_z)
            else:
                # final half slab: parallel evacuation (DVE + ACT), then
                # split the tail DMA across both hw queues.
                ot_sb = ot_sb_pool.tile([128, 2, N], F32, tag="ot_sb")
                z_sb = z_sb_pool.tile([1, N], F32, tag="z_sb")
                # z chain first on ACT (it has the longest DMA latency),
                # O^T halves split across DVE/ACT and both hw queues
                nc.scalar.copy(z_sb[0:1, 0:512], oz_il[0:1, 0:512])
                nc.vector.tensor_copy(ot_sb[:, 0, 0:512], od[0][0][:])
                nc.scalar.copy(ot_sb[:, 1, 0:512], od[1][0][:])
                nc.sync.dma_start(out=z_out[s][:n_q].unsqueeze(0),
                                  in_=z_sb[:, :n_q])
                nc.sync.dma_start(out=yt_out[s][:, 0, 0:512],
                                  in_=ot_sb[:, 0, 0:512])
                nc.scalar.dma_start(out=yt_out[s][:, 1, 0:512],
                                    in_=ot_sb[:, 1, 0:512])


_PROG_CACHE = {}


def _get_program(beta: float, fast: bool = True):
    key = (beta, fast)
    if key not in _PROG_CACHE:
        _PROG_CACHE[key] = build_program(beta, fast)
    return _PROG_CACHE[key]


def make_in_maps(x: np.ndarray, fast: bool = True):
    """Shard the full input [B, L, D, H, W] into 8 per-core input maps."""
    xt_all = np.ascontiguousarray(x.reshape(NBLK, D, N))
    in_maps = []
    for c in range(NCORES):
        half_blk = NFULL * NCORES + c // 2
        half = xt_all[half_blk]
        if c % 2 == 1:
            # rotate keys so this core's queries are columns 0..511
            half = np.concatenate([half[:, N // 2:], half[:, :N // 2]], axis=1)
        slabs = np.stack([xt_all[NFULL * c], xt_all[NFULL * c + 1], half])
        xf = np.zeros((NSLAB, N, DF), np.float32)
        xf[:, :, :D] = slabs.transpose(0, 2, 1)
        xf[:, :, D] = 1.0
        negc = -np.einsum('sdn,sdn->sn', slabs, slabs)
        # pack into device layout: xb [128, 2, N], xf [128, 8, DF]
        xb_p = slabs.reshape(NSLAB, 2, 128, N).transpose(0, 2, 1, 3)
        xf_p = xf.reshape(NSLAB, 8, 128, DF).transpose(0, 2, 1, 3)
        in_maps.append({"xb_in": np.ascontiguousarray(xb_p),
                        "xf_in": np.ascontiguousarray(xf_p),
                        "nc_in": np.ascontiguousarray(
                            negc.reshape(1, NSLAB * N))})
    return in_maps


def assemble_output(results):
    """Normalize, transpose and gather per-core outputs into [B, L, N, D]."""
    out = np.empty((NBLK, N, D), np.float32)
    for c in range(NCORES):
        yt = results[c]["yt_out"]          # [NSLAB, 128, 2, N]
        z = results[c]["z_out"]
        for s, blk, lo, n_q in ((0, NFULL * c, 0, N),
                                (1, NFULL * c + 1, 0, N),
                                (2, NFULL * NCORES + c // 2,
                                 (c % 2) * (N // 2), N // 2)):
            # O^T rows are ci*128 + p
            ot = yt[s].transpose(1, 0, 2).reshape(D, N)[:, :n_q]
            out[blk, lo:lo + n_q] = (ot / z[s, :n_q]).T
    return out.reshape(B, L, N, D)


def kernel(x, beta, _trace=False, _fast=True):
    x = np.asarray(x, dtype=np.float32)
    assert x.shape == (B, L, D, H, W), x.shape
    beta_f = float(np.asarray(beta))
    prog = _get_program(beta_f, _fast)
    in_maps = make_in_maps(x, _fast)
    res = run_bass_kernel_spmd(prog, in_maps, core_ids=list(range(NCORES)),
                               trace=_trace)
    out = assemble_output(res.results)
    if _trace:
        return out, res
    return out


# revision 23
# speedup vs baseline: 1.2142x; 1.0108x over previous
"""Trainium2 Bass kernel: batched single-head self-attention.

Reference computation (per (b, l) pair, 20 independent blocks):
    X = x[b, l] viewed as [N=1024, D=256] (xf layout)
    out[b, l] = softmax(beta * X @ X.T, axis=-1) @ X

Device algorithm (per block):
  * Scores: S[m, n] = sum_d X^T[d, m] X^T[d, n] on the TensorEngine with
    D on partitions -- the natural HBM layout of x[b, l] is already X^T.
    S is symmetric, so the PSUM tile doubles as the [keys, queries]
    orientation the second matmul wants: no transpose of the score
    matrix, ever.
  * Softmax shift: W[m, n] = exp(beta * (S[m, n] - c_n)) with
    c_n = ||x_n||^2 (the score diagonal -- a valid shift here since the
    attention is diagonal-dominant by ~100 nats). The per-QUERY shift is
    PSUM-PRELOADED: a per-slab broadcast tile bc[p, n] = -c_n (built once
    by a tiny ones x negc matmul, evacuated to SBUF) is copied by the
    DVE into each score PSUM tile before the score matmuls accumulate
    onto it (start=False). This moves the shift entirely off the
    TensorEngine stream (the old K=1 ride cost a full extra pass,
    8192 cycles/block).
  * Second matmul: computed as O^T[d, n] = sum_m xfo[m, d] W[m, n] with
    the value operand xfo = [X | 1 | 0] STATIONARY. The [1|0] chunk
    makes the softmax denominator Z_n fall out as an extra 2-row matmul
    riding the same PSUM accumulation loop (interleaved per key tile so
    there is no post-loop PE tail). Normalization (divide by Z) and the
    final [d, n] -> [n, d] layout flip happen on the host, where they
    are free.
  * Everything runs in fp32r (relaxed fp32: ~13-bit effective mantissa,
    full-rate 1 col/cycle PE streaming vs 4 cyc/col for exact fp32).
    bf16 scores are NOT enough (near-duplicate key pairs), fp32r is.

Pipelining / engine budget (per core, ~2.5 blocks):
  * PE: 2 passes scores + 2 passes O^T + 1 pass Z = 5 x 8192 cycles per
    block ~= 43 us at 2.4 GHz. Everything else hides under it.
  * ACT: exp (40 tiles x ~0.7us) + half the O^T evacuation.
  * DVE: bias preloads (40 x ~0.7us) + bc builds + Z/O^T evacuation.
  * Input DMA: slab 0's score operand is split across BOTH hw queues so
    the first matmul can start ~3 us earlier; remaining slabs stream
    under compute. Outputs go out on the gpsimd queue (s<2) and split
    across the sync+scalar queues for the final half slab (short tail).
  * PE p-state ramp: a few throwaway full-width fp32r matmuls run
    during the input-DMA window so the clock ramp (~3 us) mostly
    completes before real data arrives.

Sharding: 20 blocks over 8 cores as 2 full blocks + 1 half block (512
queries) per core -- exact, no padded compute. The half blocks use a
host-side rotation of the key axis so every core runs the identical
program (softmax is invariant to key permutation when values are
permuted identically).
"""

import numpy as np
import ml_dtypes

import concourse.tile as tile
from concourse import bacc, mybir
from concourse.bass_utils import run_bass_kernel_spmd

F32 = mybir.dt.float32
F32R = mybir.dt.float32r
BF16 = mybir.dt.bfloat16

B, L, D, H, W = 4, 5, 256, 32, 32
N = H * W            # 1024 keys per block
NBLK = B * L         # 20
NCORES = 8
NFULL = 2            # full blocks per core
NSLAB = 3            # 2 full + 1 half
DF = D + 8           # value operand row: [x | 1 | 0 | pad...] -- padded to
                     # 264 floats = 1056 B so SBUF rows stay 32B-aligned
                     # (unaligned weight rows double LDWEIGHTS time)
NWARM = 1            # PE clock-ramp warmup matmuls (the bias
                     # builds continue the ramp with real work)

EXP = mybir.ActivationFunctionType.Exp


def build_program(beta: float, fast: bool = True):
    mdt = F32R if fast else F32   # all matmul operands
    nc = bacc.Bacc("TRN2", target_bir_lowering=False, debug=False,
                   num_devices=NCORES)
    # Inputs are host-packed in device layout so every DMA is a plain
    # contiguous [128, *] transfer with large descriptors.
    xb_in = nc.dram_tensor("xb_in", [NSLAB, 128, 2, N], mdt,
                           kind="ExternalInput")
    xf_in = nc.dram_tensor("xf_in", [NSLAB, 128, 8, DF], mdt,
                           kind="ExternalInput")
    nc_in = nc.dram_tensor("nc_in", [1, NSLAB * N], mdt, kind="ExternalInput")
    yt_out = nc.dram_tensor("yt_out", [NSLAB, 128, 2, N], F32,
                            kind="ExternalOutput")
    z_out = nc.dram_tensor("z_out", [NSLAB, N], F32, kind="ExternalOutput")

    with tile.TileContext(nc) as tc:
        _build(tc, nc, xb_in.ap(), xf_in.ap(), nc_in.ap(), yt_out.ap(),
               z_out.ap(), beta, mdt)
    nc.finalize()
    return nc


def _build(tc, nc, xb_in, xf_in, nc_in, yt_out, z_out, beta, mdt):
    import contextlib
    ctx = contextlib.ExitStack()
    with ctx:
        const = ctx.enter_context(tc.tile_pool(name="const", bufs=1))
        xb_pool = ctx.enter_context(tc.tile_pool(name="xb", bufs=NSLAB))
        xfo_pool = ctx.enter_context(tc.tile_pool(name="xfo", bufs=NSLAB))
        negc_pool = ctx.enter_context(tc.tile_pool(name="negc", bufs=1))
        bc_pool = ctx.enter_context(tc.tile_pool(name="bc", bufs=NSLAB))
        # W tiles stay live until the post-loop Z pass of their slab.
        w_pool = ctx.enter_context(tc.tile_pool(name="w", bufs=9))
        ot_sb_pool = ctx.enter_context(tc.tile_pool(name="ot_sb", bufs=2))
        z_sb_pool = ctx.enter_context(tc.tile_pool(name="z_sb", bufs=2))
        # PSUM budget (8 banks of [128, 512] f32):
        #   4 score tiles + 4 O^T accumulators. The bias preloads get a
        #   full key-tile of slack from the 4-deep score rotation; the Z
        #   row-sum pass runs post-loop in freed score banks (full
        #   slabs) or interleaved in a spare O^T slot (final half slab).
        ps_s = ctx.enter_context(tc.tile_pool(name="ps_s", bufs=4, space="PSUM"))
        ps_od = ctx.enter_context(tc.tile_pool(name="ps_od", bufs=4, space="PSUM"))


        # Warm the PE clock (HAM) with throwaway full-array matmuls that
        # run during the input-DMA window -- otherwise the first ~3us of
        # real matmuls run at half clock.
        warm_src = const.tile([128, 512], F32)
        nc.gpsimd.memset(warm_src[:], 0.0)
        warm_ps = ps_od.tile([128, 512], F32, tag="od", name="warm_ps")
        for wi in range(NWARM):
            nc.tensor.matmul(warm_ps[:], warm_src[:, 0:128], warm_src[:],
                             start=True, stop=True)

        # Input DMAs: slab 0's score operand is split across both hw
        # queues (earliest possible first matmul); the rest interleave so
        # each slab's operands land well before its compute window. The
        # bias broadcast tiles bc[p, n] = -c_n are built by partition-
        # broadcast DMAs (stride-0 source) on the otherwise-idle gpsimd
        # queue -- no compute engine touches them.
        xbs = [xb_pool.tile([128, 2, N], mdt, tag="xb", name=f"xb_{s}")
               for s in range(NSLAB)]
        xfos = [xfo_pool.tile([128, 8, DF], mdt, tag="xfo", name=f"xfo_{s}")
                for s in range(NSLAB)]
        bcs = [bc_pool.tile([128, N], F32, tag="bc", name=f"bc_{s}")
               for s in range(NSLAB)]
        negc_all = negc_pool.tile([1, NSLAB * N], mdt, tag="negc")
        # sync queue (starts flowing ~2us earlier than scalar; carries the
        # bias row -- which gates the first preload -- then chunk 0 and
        # the high-column half of chunk 1)
        nc.sync.dma_start(out=negc_all[:], in_=nc_in[:])
        nc.sync.dma_start(out=xbs[0][:, 0:1, :], in_=xb_in[0][:, 0:1, :])
        nc.sync.dma_start(out=xfos[0][:, 0:4, :], in_=xf_in[0][:, 0:4, :])
        nc.sync.dma_start(out=xbs[1][:], in_=xb_in[1])
        nc.sync.dma_start(out=xbs[2][:], in_=xb_in[2])
        # scalar queue
        nc.scalar.dma_start(out=xbs[0][:, 1:2, :], in_=xb_in[0][:, 1:2, :])
        nc.scalar.dma_start(out=xfos[0][:, 4:8, :], in_=xf_in[0][:, 4:8, :])
        nc.scalar.dma_start(out=xfos[1][:], in_=xf_in[1])
        nc.scalar.dma_start(out=xfos[2][:], in_=xf_in[2])

        ones_col_f32 = const.tile([1, 128], F32)
        nc.gpsimd.memset(ones_col_f32[:], 1.0)
        if mdt is F32:
            ones_col = ones_col_f32
        else:
            ones_col = const.tile([1, 128], mdt)
            nc.vector.tensor_copy(ones_col[:], ones_col_f32[:])

        # Bias broadcast tiles bc[p, n] = -c_n for all slabs, built
        # upfront during the DMA window: K=1 ones x negc matmuls into
        # transient score-pool banks, evacuated to SBUF by the DVE (the
        # first preload chains directly off the slab-0 h0 copy).
        for s in range(NSLAB):
            for h in range(2 if s < NFULL else 1):
                hs = slice(h * 512, (h + 1) * 512)
                bc_ps = ps_s.tile([128, 512], F32, tag="sps",
                                  name=f"bcps_{s}_{h}")
                nc.tensor.matmul(bc_ps[:], ones_col[:],
                                 negc_all[:, s * N + h * 512:
                                          s * N + (h + 1) * 512],
                                 start=True, stop=True)
                nc.vector.tensor_copy(bcs[s][:, hs], bc_ps[:])

        pending_z = []
        for s in range(NSLAB):
            n_q = N if s < NFULL else N // 2
            n_h = n_q // 512    # PSUM bank halves (queries)
            xb, xfo, bc = xbs[s], xfos[s], bcs[s]

            # O^T accumulators, live across the whole key loop; the
            # final half slab also carves a Z accumulator out of the
            # spare O^T slots.
            od = [[ps_od.tile([128, 512], F32, tag="od",
                              name=f"od_{s}_{ci}_{h}")
                   for h in range(n_h)] for ci in range(2)]
            oz_il = (None if s < NFULL else
                     ps_od.tile([128, 512], F32, tag="od", name=f"oz_{s}"))

            w_tiles = []
            for a in range(8):      # key tile (partitions of S' and W)
                if a == 1 and pending_z:
                    # previous slab's Z pass, deferred past this slab's
                    # first key tile: its scores+exp fill the PE->ACT
                    # pipeline while the Z matmuls run, so the slab
                    # boundary has no drain bubble
                    for fn in pending_z:
                        fn()
                    pending_z = []
                asl = slice(a * 128, (a + 1) * 128)
                wt = w_pool.tile([128, N], mdt, tag="w", name=f"w_{s}_{a}")
                sps = []
                for h in range(n_h):
                    t = ps_s.tile([128, 512], F32, tag="sps",
                                  name=f"sps_{s}_{a}_{h}")
                    # bias preload: PSUM starts at -c_n, scores accumulate
                    nc.vector.tensor_copy(t[:], bc[:, h * 512:(h + 1) * 512])
                    sps.append(t)
                # S' = -c_n + S, per query half; order c0,c1 within a half
                # so exp(h) can issue before the other half's matmuls.
                for h in range(n_h):
                    hs = slice(h * 512, (h + 1) * 512)
                    for c in range(2):
                        nc.tensor.matmul(sps[h][:], xb[:, c, asl],
                                         xb[:, c, hs],
                                         start=False, stop=(c == 1),
                                         skip_group_check=True)
                    # W = exp(beta * S'), one ACT pass per query half
                    nc.scalar.activation(wt[:, hs], sps[h][:], EXP,
                                         scale=float(beta))
                # final half slab: Z rides the key loop (no PE tail);
                # it goes first so the z evacuation chain starts earliest
                if s >= NFULL:
                    nc.tensor.matmul(oz_il[0:2, 0:512],
                                     xfo[:, a, 256:258], wt[:, 0:512],
                                     start=(a == 0), stop=(a == 7))
                # O^T += xfo[a].T @ W[a]  (value operand stationary)
                for ci, csl in ((0, slice(0, 128)), (1, slice(128, 256))):
                    for h in range(n_h):
                        hs = slice(h * 512, (h + 1) * 512)
                        nc.tensor.matmul(od[ci][h][:], xfo[:, a, csl],
                                         wt[:, hs],
                                         start=(a == 0), stop=(a == 7))
                w_tiles.append(wt)

            if s < NFULL:
                # O^T evacuation (DVE + ACT in parallel) and shipping
                # happen now; the Z pass (row sums of W in freed score
                # banks) is deferred into the next slab's stream.
                ot_sb = ot_sb_pool.tile([128, 2, N], F32, tag="ot_sb")
                for h in range(n_h):
                    hs = slice(h * 512, (h + 1) * 512)
                    nc.vector.tensor_copy(ot_sb[:, 0, hs], od[0][h][:])
                    nc.scalar.copy(ot_sb[:, 1, hs], od[1][h][:])
                nc.gpsimd.dma_start(out=yt_out[s], in_=ot_sb[:])

                def _flush_z(s=s, xfo=xfo, w_tiles=w_tiles, n_q=n_q,
                             n_h=n_h):
                    z_sb = z_sb_pool.tile([1, N], F32, tag="z_sb")
                    oz = [ps_s.tile([128, 512], F32, tag="sps",
                                    name=f"oz_{s}_{h}")
                          for h in range(n_h)]
                    for a in range(8):
                        for h in range(n_h):
                            hs = slice(h * 512, (h + 1) * 512)
                            nc.tensor.matmul(oz[h][0:2, 0:512],
                                             xfo[:, a, 256:258],
                                             w_tiles[a][:, hs],
                                             start=(a == 0), stop=(a == 7))
                    for h in range(n_h):
                        hs = slice(h * 512, (h + 1) * 512)
                        nc.vector.tensor_copy(z_sb[:, hs],
                                              oz[h][0:1, 0:512])
                    nc.gpsimd.dma_start(out=z_out[s][:n_q].unsqueeze(0),
                                        in_=z_sb[:, :n_q])
                pending_z.append(_flush_z)
            else:
                # final half slab: parallel evacuation (DVE + ACT), then
                # split the tail DMA across both hw queues.
                ot_sb = ot_sb_pool.tile([128, 2, N], F32, tag="ot_sb")
                z_sb = z_sb_pool.tile([1, N], F32, tag="z_sb")
                # z chain first on ACT (it has the longest DMA latency),
                # O^T halves split across DVE/ACT and both hw queues
                nc.scalar.copy(z_sb[0:1, 0:512], oz_il[0:1, 0:512])
                nc.vector.tensor_copy(ot_sb[:, 0, 0:512], od[0][0][:])
                nc.scalar.copy(ot_sb[:, 1, 0:512], od[1][0][:])
                nc.sync.dma_start(out=z_out[s][:n_q].unsqueeze(0),
                                  in_=z_sb[:, :n_q])
                nc.sync.dma_start(out=yt_out[s][:, 0, 0:512],
                                  in_=ot_sb[:, 0, 0:512])
                nc.scalar.dma_start(out=yt_out[s][:, 1, 0:512],
                                    in_=ot_sb[:, 1, 0:512])


_PROG_CACHE = {}


def _get_program(beta: float, fast: bool = True):
    key = (beta, fast)
    if key not in _PROG_CACHE:
        _PROG_CACHE[key] = build_program(beta, fast)
    return _PROG_CACHE[key]


def make_in_maps(x: np.ndarray, fast: bool = True):
    """Shard the full input [B, L, D, H, W] into 8 per-core input maps."""
    xt_all = np.ascontiguousarray(x.reshape(NBLK, D, N))
    in_maps = []
    for c in range(NCORES):
        half_blk = NFULL * NCORES + c // 2
        half = xt_all[half_blk]
        if c % 2 == 1:
            # rotate keys so this core's queries are columns 0..511
            half = np.concatenate([half[:, N // 2:], half[:, :N // 2]], axis=1)
        slabs = np.stack([xt_all[NFULL * c], xt_all[NFULL * c + 1], half])
        xf = np.zeros((NSLAB, N, DF), np.float32)
        xf[:, :, :D] = slabs.transpose(0, 2, 1)
        xf[:, :, D] = 1.0
        negc = -np.einsum('sdn,sdn->sn', slabs, slabs)
        # pack into device layout: xb [128, 2, N], xf [128, 8, DF]
        xb_p = slabs.reshape(NSLAB, 2, 128, N).transpose(0, 2, 1, 3)
        xf_p = xf.reshape(NSLAB, 8, 128, DF).transpose(0, 2, 1, 3)
        in_maps.append({"xb_in": np.ascontiguousarray(xb_p),
                        "xf_in": np.ascontiguousarray(xf_p),
                        "nc_in": np.ascontiguousarray(
                            negc.reshape(1, NSLAB * N))})
    return in_maps


def assemble_output(results):
    """Normalize, transpose and gather per-core outputs into [B, L, N, D]."""
    out = np.empty((NBLK, N, D), np.float32)
    for c in range(NCORES):
        yt = results[c]["yt_out"]          # [NSLAB, 128, 2, N]
        z = results[c]["z_out"]
        for s, blk, lo, n_q in ((0, NFULL * c, 0, N),
                                (1, NFULL * c + 1, 0, N),
                                (2, NFULL * NCORES + c // 2,
                                 (c % 2) * (N // 2), N // 2)):
            # O^T rows are ci*128 + p
            ot = yt[s].transpose(1, 0, 2).reshape(D, N)[:, :n_q]
            out[blk, lo:lo + n_q] = (ot / z[s, :n_q]).T
    return out.reshape(B, L, N, D)


def kernel(x, beta, _trace=False, _fast=True):
    x = np.asarray(x, dtype=np.float32)
    assert x.shape == (B, L, D, H, W), x.shape
    beta_f = float(np.asarray(beta))
    prog = _get_program(beta_f, _fast)
    in_maps = make_in_maps(x, _fast)
    res = run_bass_kernel_spmd(prog, in_maps, core_ids=list(range(NCORES)),
                               trace=_trace)
    out = assemble_output(res.results)
    if _trace:
        return out, res
    return out
